# revision 15
# baseline (speedup 1.0000x reference)
"""TAGConvNet (2x TAGConv K=3 + MLP) on 8 trn2 NeuronCores via Bass/Tile.

v2: flipped-orientation bf16 message passing.
- Node-partition across 8 cores (12544 padded rows each, 98 blocks of 128).
- Table = x_k row-major bf16 [NTOT,128] (replicated via AllGather each hop).
- Per hop: dma_gather rows of the table (bf16), scatter via matmul with
  STATIC one-hot tiles (norm_e folded in, loaded from DRAM):
      accT[C, tgt] += msg[e, C]^T-as-lhsT @ oh[e, tgt]
  giving x_{k+1} directly transposed for the dense W matmuls. Table writes
  go through the DMA XBAR transpose. Buckets are padded to 16-idx
  granularity (cross-core max); scatter matmuls are partition-sliced
  "pieces" so chunks may mix adjacent blocks of a 4-block group.
"""
import sys
from contextlib import ExitStack

import numpy as np

sys.path.insert(0, "/opt/trn_rl_repo")

import concourse.bass as bass  # noqa: E402
import concourse.tile as tile  # noqa: E402
from concourse import bacc, mybir  # noqa: E402
from concourse.bass_utils import run_bass_kernel_spmd  # noqa: E402

P = 8                 # cores
NBLK = 98             # 128-node blocks per core
NB = NBLK * 128       # 12544 padded nodes per core
NTOT = P * NB         # 100352
HA = 6144             # A-half shard rows (blocks 0-47)
HB = 6400             # B-half shard rows (blocks 48-97)
SEGA = 24576          # A table seg size (8*HA/2)
SEGB = 25600          # B table seg size (8*HB/2)
NSEGS = 4
GBLK = 4              # blocks per psum group
MAXL = 2048           # max idxs per dma_gather call
PAD_GRAN = 16         # bucket padding granularity (128 = chunk-aligned)
DT = mybir.dt

_cache = {}


def _host_prep(edge_index, n_real):
    """Bucket edges by (core, target block, source segment); pad buckets to
    PAD_GRAN (cross-core common). Returns per-core idx streams, static
    one-hot tiles (norm folded), and the call/piece plan."""
    npc = n_real // P
    row, col = edge_index[0].astype(np.int64), edge_index[1].astype(np.int64)

    deg = np.bincount(col, minlength=n_real)
    dis = np.where(deg > 0, 1.0 / np.sqrt(np.maximum(deg, 1.0)), 0.0).astype(np.float64)
    norm = (dis[row] * dis[col]).astype(np.float32)  # [E]

    def to_gid(i):
        return (i // npc) * NB + (i % npc)

    rg, cg = to_gid(row), to_gid(col)
    core = cg // NB
    loc = cg - core * NB
    blk = loc >> 7
    slot = loc & 127
    # A/B half-shard tables: A = shard rows [0,6144), B = [6144,12544).
    # Table rows: A: c*6144+j (2 segs of 24576); B: c*6400+(j-6144) (2 of 25600)
    c_s, j_s = row // npc, row % npc
    in_a = j_s < HA
    arow = c_s * HA + j_s
    brow = c_s * HB + (j_s - HA)
    seg = np.where(in_a, arow // SEGA, 2 + brow // SEGB)
    rel = np.where(in_a, arow % SEGA, brow % SEGB)

    cnt = np.zeros((P, NBLK, NSEGS), np.int64)
    np.add.at(cnt, (core, blk, seg), 1)
    gran = PAD_GRAN
    pbs = (gran * np.ceil(cnt.max(axis=0) / gran)).astype(np.int64)  # [NBLK, NSEGS]

    # stream layout: per group of GBLK blocks: per seg: buckets back-to-back.
    # calls: split at MAXL (16-granular) within (group, seg).
    off = np.zeros((NBLK, NSEGS), np.int64)
    pos = 0
    groups = [list(range(g, min(g + GBLK, NBLK))) for g in range(0, NBLK, GBLK)]
    calls = []  # (gi, stream_off, L, seg, [(block, off_in_call, len), ...])
    for gi, blocks in enumerate(groups):
        for s in range(NSEGS):
            cur = None
            for b in blocks:
                n = int(pbs[b, s])
                if n == 0:
                    continue
                if cur is not None and cur[2] + n <= MAXL:
                    off[b, s] = pos
                    cur[4].append((b, cur[2], n))
                    cur[2] += n
                    pos += n
                else:
                    if cur is not None:
                        calls.append(tuple(cur))
                        pos = cur[1] + 128 * (-(-cur[2] // 128))
                    cur = [gi, pos, n, s, [(b, 0, n)]]
                    off[b, s] = pos
                    pos += n
            if cur is not None:
                calls.append(tuple(cur))
                pos = cur[1] + 128 * (-(-cur[2] // 128))
    epad = pos

    # per-call pieces; one-hot tiles are per-PIECE (rows outside the piece
    # zeroed) so every matmul uses the full 128-row chunk as lhsT.
    totch = 0
    call_plan = []  # (gi, c_off, L, s, pid0, pieces)
    for (gi, c_off, L, s, bks) in calls:
        nch = -(-L // 128)
        pieces = []  # (ch, r0, r1, b)
        for (b, o, n) in bks:
            p0 = o
            while p0 < o + n:
                ch = p0 // 128
                p1 = min(o + n, (ch + 1) * 128)
                pieces.append((ch, p0 - ch * 128, p1 - ch * 128, b))
                p0 = p1
        call_plan.append((gi, c_off, L, s, totch, pieces))
        totch += len(pieces)

    # first/last piece flags per block (per hop program order)
    first_piece = {}
    last_piece = {}
    for ci, (gi, c_off, L, s, pid0, pieces) in enumerate(call_plan):
        for pi, (ch, r0, r1, b) in enumerate(pieces):
            if b not in first_piece:
                first_piece[b] = (ci, pi)
            last_piece[b] = (ci, pi)
    assert len(first_piece) == NBLK, "every block needs at least one edge bucket"
    # PSUM bank packing: 4 accs per 2KB bank; start=True only on the
    # program-first matmul touching each bank (zeroes the whole bank).
    bank_first = {}
    bank_last = {}
    for gi, blocks in enumerate(groups):
        for bk in range(0, len(blocks), 4):
            bank_blocks = blocks[bk:bk + 4]
            bank_first[(gi, bk // 4)] = min(first_piece[b] for b in bank_blocks)
            bank_last[(gi, bk // 4)] = max(last_piece[b] for b in bank_blocks)

    # per-core idx streams + one-hot tiles
    key = (core * NBLK + blk) * NSEGS + seg
    order = np.argsort(key, kind="stable")
    key_s = key[order]
    first = np.searchsorted(key_s, key_s)
    rank = np.arange(len(key_s)) - first
    dst = off[blk[order], seg[order]] + rank  # stream position

    # pad the idx stream so every call can gather whole 128-row chunks
    # (rows past a call's L are oh-masked; extra idxs read the next call's
    # stream region, which is always a valid row of the current segment)
    epad_pad = epad + MAXL
    gidx = np.zeros((P, epad_pad), np.int16)
    gidx[core[order], dst] = rel[order].astype(np.int16)
    idx16 = np.tile(gidx.reshape(P, epad_pad // 16, 16).transpose(0, 2, 1),
                    (1, 8, 1)).copy()

    # one-hot tiles [128, totpieces, 128] per core: oh[row, pid, slot] = norm
    # for rows within the piece's [r0, r1) chunk-row range, zero elsewhere.
    pos2pid = np.zeros(epad, np.int64)
    pos2row = np.zeros(epad, np.int64)
    for (gi, c_off, L, s, pid0, pieces) in call_plan:
        for pi, (ch, r0, r1, b) in enumerate(pieces):
            a = c_off + ch * 128 + r0
            n = r1 - r0
            pos2pid[a:a + n] = pid0 + pi
            pos2row[a:a + n] = np.arange(r0, r1)
    oh = np.zeros((P, 128, totch, 128), np.float32)
    oh[core[order], pos2row[dst], pos2pid[dst], slot[order]] = norm[order]

    return dict(epad=epad_pad, call_plan=call_plan, idx16=idx16, oh=oh,
                first_piece=first_piece, last_piece=last_piece,
                bank_first=bank_first, bank_last=bank_last,
                groups=groups, totch=totch, npc=npc)


def _build(prep, n_g, k_hops, n_m):
    epad = prep["epad"]
    call_plan = prep["call_plan"]
    groups = prep["groups"]
    totch = prep["totch"]
    first_piece, last_piece = prep["first_piece"], prep["last_piece"]
    bank_first = prep["bank_first"]
    bank_last = prep["bank_last"]
    max_pieces = max(len(p[5]) for p in call_plan)
    nm1 = k_hops + 1

    nc = bacc.Bacc("TRN2", target_bir_lowering=False, debug=False, num_devices=P)

    xT_d = nc.dram_tensor("xT", [8, NB], DT.float32, kind="ExternalInput")
    idx_d = nc.dram_tensor("idx", [128, epad // 16], DT.int16, kind="ExternalInput")
    oh_d = nc.dram_tensor("oh", [128, totch, 128], DT.bfloat16, kind="ExternalInput")
    w0_d = nc.dram_tensor("w0", [8, 128], DT.float32, kind="ExternalInput")
    b0_d = nc.dram_tensor("b0", [128, 1], DT.float32, kind="ExternalInput")
    wtag_d = nc.dram_tensor("wtag", [n_g * nm1, 128, 128], DT.float32, kind="ExternalInput")
    wtagb_d = nc.dram_tensor("wtagb", [n_g * nm1, 128, 128], DT.bfloat16, kind="ExternalInput")
    btag_d = nc.dram_tensor("btag", [128, n_g], DT.float32, kind="ExternalInput")
    wmlp_d = nc.dram_tensor("wmlp", [n_m, 128, 128], DT.float32, kind="ExternalInput")
    bmlp_d = nc.dram_tensor("bmlp", [128, n_m], DT.float32, kind="ExternalInput")
    w1_d = nc.dram_tensor("w1", [128, 1], DT.float32, kind="ExternalInput")
    b1_d = nc.dram_tensor("b1", [1, 1], DT.float32, kind="ExternalInput")
    y_d = nc.dram_tensor("y", [1, NB], DT.float32, kind="ExternalOutput")

    xinA = [nc.dram_tensor(f"xinA{i}", [HA, 128], DT.bfloat16) for i in range(2)]
    xinB = [nc.dram_tensor(f"xinB{i}", [HB, 128], DT.bfloat16) for i in range(2)]
    xtabA = [nc.dram_tensor(f"xtabA{i}", [P * HA, 128], DT.bfloat16,
                            addr_space="Shared") for i in range(2)]
    xtabB = [nc.dram_tensor(f"xtabB{i}", [P * HB, 128], DT.bfloat16,
                            addr_space="Shared") for i in range(2)]
    rg = [list(range(P))]

    with tile.TileContext(nc) as tc:
        with ExitStack() as ctx:
            const = ctx.enter_context(tc.tile_pool(name="const", bufs=1))
            big = ctx.enter_context(tc.tile_pool(name="big", bufs=1))
            mpool = ctx.enter_context(tc.tile_pool(name="msg", bufs=3))
            opool = ctx.enter_context(tc.tile_pool(name="oh", bufs=2))
            wpool = ctx.enter_context(tc.tile_pool(name="work", bufs=2))
            tpool = ctx.enter_context(tc.tile_pool(name="tr", bufs=2))
            pacc = ctx.enter_context(tc.tile_pool(name="pacc", bufs=1, space="PSUM"))
            pden = ctx.enter_context(tc.tile_pool(name="pden", bufs=2, space="PSUM"))

            idx_sb = const.tile([128, epad // 16], DT.int16)
            nc.sync.dma_start(idx_sb[:], idx_d[:])

            w0_sb = const.tile([8, 128], DT.float32)
            nc.sync.dma_start(w0_sb[:], w0_d[:])
            b0_sb = const.tile([128, 1], DT.float32)
            nc.sync.dma_start(b0_sb[:], b0_d[:])
            wtagb_sb = []
            for i in range(n_g * nm1):
                tb = const.tile([128, 128], DT.bfloat16, tag=f"wtagb{i}")
                nc.sync.dma_start(tb[:], wtagb_d[i])
                wtagb_sb.append(tb)
            btag_sb = const.tile([128, n_g], DT.float32)
            nc.sync.dma_start(btag_sb[:], btag_d[:])
            wmlp_sb = []
            for i in range(n_m):
                t = const.tile([128, 128], DT.float32, tag=f"wmlp{i}")
                nc.sync.dma_start(t[:], wmlp_d[i])
                wmlp_sb.append(t)
            bmlp_sb = const.tile([128, n_m], DT.float32)
            nc.sync.dma_start(bmlp_sb[:], bmlp_d[:])
            w1_sb = const.tile([128, 1], DT.float32)
            nc.sync.dma_start(w1_sb[:], w1_d[:])
            b1_sb = const.tile([1, 1], DT.float32)
            nc.sync.dma_start(b1_sb[:], b1_d[:])

            hT = big.tile([128, NB], DT.float32)    # h transposed [C, nodes]
            oT = big.tile([128, NB], DT.float32)    # out accumulator

            cpy = mybir.ActivationFunctionType.Copy
            rel = mybir.ActivationFunctionType.Relu

            def table_dst(bb, w, slot):
                if bb < 48:
                    t = xinA[slot][128 * bb:128 * bb + w, :]
                else:
                    t = xinB[slot][128 * (bb - 48):128 * (bb - 48) + w, :]
                return t.rearrange("(c p) f -> p c f", p=128)

            def ag_a(slot):
                nc.gpsimd.collective_compute(
                    "AllGather", mybir.AluOpType.bypass, replica_groups=rg,
                    ins=[xinA[slot][:]], outs=[xtabA[slot][:]])

            def ag_b(slot):
                nc.gpsimd.collective_compute(
                    "AllGather", mybir.AluOpType.bypass, replica_groups=rg,
                    ins=[xinB[slot][:]], outs=[xtabB[slot][:]])

            def write_table(src_sb, slot, need_convert):
                for bb in range(0, NBLK, 4):
                    nb4 = min(4, NBLK - bb)
                    w = nb4 * 128
                    sl = src_sb[:, 128 * bb:128 * bb + w]
                    if need_convert:
                        cb = wpool.tile([128, 512], DT.bfloat16, tag="cb")
                        nc.scalar.activation(cb[:, :w], sl, cpy)
                        sl = cb[:, :w]
                    tr = tpool.tile([128, GBLK, 128], DT.bfloat16, tag="tr")
                    nc.sync.dma_start_transpose(tr[:, :nb4, :], sl)
                    nc.sync.dma_start(table_dst(bb, w, slot), tr[:, :nb4, :])
                    if bb + 4 == 48:
                        ag_a(slot)
                ag_b(slot)

            # ---- lin0: hT = relu(W0^T xT + b0) ----
            for bb in range(0, NBLK, 4):
                w = min(4, NBLK - bb) * 128
                xt = wpool.tile([8, 512], DT.float32, tag="xt")
                nc.sync.dma_start(xt[:, :w], xT_d[:, 128 * bb:128 * bb + w])
                ph = pden.tile([128, 512], DT.float32, tag="ph")
                nc.tensor.matmul(ph[:, :w], w0_sb[:], xt[:, :w])
                nc.scalar.activation(hT[:, 128 * bb:128 * bb + w], ph[:, :w],
                                     rel, bias=b0_sb[:])

            par = 0
            write_table(hT, par, need_convert=True)

            for g in range(n_g):
                # out = W[g,0]^T h (bf16 weights; convert h slices)
                for bb in range(0, NBLK, 4):
                    w = min(4, NBLK - bb) * 128
                    hb = wpool.tile([128, 512], DT.bfloat16, tag="cb")
                    nc.scalar.activation(hb[:, :w], hT[:, 128 * bb:128 * bb + w], cpy)
                    po = pden.tile([128, 512], DT.float32, tag="ph")
                    nc.tensor.matmul(po[:, :w], wtagb_sb[g * nm1][:], hb[:, :w])
                    nc.vector.tensor_copy(oT[:, 128 * bb:128 * bb + w], po[:, :w])

                for k in range(1, k_hops + 1):
                    nxt = par ^ 1
                    ci = 0
                    for gi, blocks in enumerate(groups):
                        accs = {b: pacc.tile([128, 128], DT.float32,
                                             name=f"acc_{g}_{k}_{b}",
                                             tag=f"acc{b - blocks[0]}")
                                for b in blocks}

                        def acc_ap(b):
                            return accs[b][:]

                        while ci < len(call_plan) and call_plan[ci][0] == gi:
                            (_, c_off, L, s, pid0, pieces) = call_plan[ci]
                            nch = -(-L // 128)
                            npc_ = len(pieces)
                            msg = mpool.tile([128, MAXL // 128, 128], DT.bfloat16,
                                             tag="msg")
                            lg = nch * 128
                            if s < 2:
                                src_ap = xtabA[par][s * SEGA:(s + 1) * SEGA, :]
                            else:
                                src_ap = xtabB[par][(s - 2) * SEGB:(s - 1) * SEGB, :]
                            nc.gpsimd.dma_gather(
                                out_ap=msg[:, :nch, :],
                                in_ap=src_ap,
                                idxs_ap=idx_sb[:, c_off // 16:(c_off + lg) // 16],
                                num_idxs=lg, num_idxs_reg=lg, elem_size=128)
                            oht = opool.tile([128, max_pieces, 128],
                                             DT.bfloat16, tag="oht")
                            nc.sync.dma_start(oht[:, :npc_, :],
                                              oh_d[:, pid0:pid0 + npc_, :])
                            for pi, (ch, r0, r1, b) in enumerate(pieces):
                                nc.tensor.matmul(
                                    acc_ap(b),
                                    msg[:, ch, :], oht[:, pi, :],
                                    start=(first_piece[b] == (ci, pi)),
                                    stop=(last_piece[b] == (ci, pi)))
                            ci += 1
                        # group finalize: xcur tile, dense W_k, table write
                        bb = blocks[0]
                        w = len(blocks) * 128
                        xc = wpool.tile([128, GBLK * 128], DT.bfloat16, tag="xc")
                        for j, b in enumerate(blocks):
                            nc.scalar.activation(xc[:, 128 * j:128 * (j + 1)],
                                                 acc_ap(b), cpy)
                        for dd in range(0, w, 512):
                            dw = min(512, w - dd)
                            po = pden.tile([128, 512], DT.float32, tag="ph")
                            nc.tensor.matmul(po[:, :dw], wtagb_sb[g * nm1 + k][:],
                                             xc[:, dd:dd + dw])
                            nc.vector.tensor_add(
                                oT[:, 128 * bb + dd:128 * bb + dd + dw],
                                oT[:, 128 * bb + dd:128 * bb + dd + dw], po[:, :dw])
                        if k < k_hops:
                            tr = tpool.tile([128, GBLK, 128], DT.bfloat16, tag="tr")
                            nc.sync.dma_start_transpose(
                                tr[:, :len(blocks), :], xc[:, :w])
                            nc.sync.dma_start(table_dst(bb, w, nxt),
                                              tr[:, :len(blocks), :])
                            if 128 * bb + w == 128 * 48:
                                ag_a(nxt)
                    if k < k_hops:
                        ag_b(nxt)
                        par = nxt

                # layer end: h = relu(out + b_tag[g])
                nc.scalar.activation(oT[:], oT[:], rel, bias=btag_sb[:, g:g + 1])
                hT, oT = oT, hT
                if g < n_g - 1:
                    nxt = par ^ 1
                    write_table(hT, nxt, need_convert=True)
                    par = nxt

            # ---- MLP ----
            for m in range(n_m):
                for bb in range(0, NBLK, 4):
                    w = min(4, NBLK - bb) * 128
                    po = pden.tile([128, 512], DT.float32, tag="ph")
                    nc.tensor.matmul(po[:, :w], wmlp_sb[m][:],
                                     hT[:, 128 * bb:128 * bb + w])
                    nc.scalar.activation(oT[:, 128 * bb:128 * bb + w], po[:, :w],
                                         rel, bias=bmlp_sb[:, m:m + 1])
                hT, oT = oT, hT

            # ---- head ----
            ysb = big.tile([1, NB], DT.float32)
            for bb in range(0, NBLK, 4):
                w = min(4, NBLK - bb) * 128
                py = pden.tile([128, 512], DT.float32, tag="ph")
                nc.tensor.matmul(py[:1, :w], w1_sb[:], hT[:, 128 * bb:128 * bb + w])
                nc.scalar.activation(ysb[:, 128 * bb:128 * bb + w], py[:1, :w],
                                     rel, bias=b1_sb[:])
            nc.sync.dma_start(y_d[:], ysb[:])

    nc.compile()
    return nc


def _setup(x, edge_index, W0, b0, W_tag, b_tag, W_mlp, b_mlp, W1, b1):
    import jax.numpy as jnp
    x = np.asarray(x, np.float32)
    edge_index = np.asarray(edge_index)
    n_real = x.shape[0]
    n_g, nm1 = W_tag.shape[0], W_tag.shape[1]
    n_m = W_mlp.shape[0]

    ck = (n_real, edge_index.shape[1], int(edge_index[0, ::997].astype(np.int64).sum()),
          int(edge_index[1, ::997].astype(np.int64).sum()))
    if ck not in _cache:
        prep = _host_prep(edge_index, n_real)
        nc = _build(prep, n_g, nm1 - 1, n_m)
        oh_bf = np.asarray(jnp.asarray(prep["oh"], dtype=jnp.bfloat16))
        _cache[ck] = (prep, nc, oh_bf)
    prep, nc, oh_bf = _cache[ck]

    npc = prep["npc"]
    xT = np.zeros((P, 8, NB), np.float32)
    xs = x.reshape(P, npc, -1)
    for c in range(P):
        xT[c, :xs.shape[2], :npc] = xs[c].T

    wtag = np.ascontiguousarray(W_tag.reshape(n_g * nm1, 128, 128), dtype=np.float32)
    wtagb = np.asarray(jnp.asarray(wtag, dtype=jnp.bfloat16))
    in_maps = []
    for c in range(P):
        in_maps.append({
            "xT": xT[c], "idx": prep["idx16"][c], "oh": oh_bf[c],
            "w0": np.vstack([np.asarray(W0, np.float32),
                             np.zeros((8 - W0.shape[0], 128), np.float32)]),
            "b0": np.asarray(b0, np.float32).reshape(128, 1),
            "wtag": wtag, "wtagb": wtagb,
            "btag": np.ascontiguousarray(np.asarray(b_tag, np.float32).T),
            "wmlp": np.asarray(W_mlp, np.float32),
            "bmlp": np.ascontiguousarray(np.asarray(b_mlp, np.float32).T),
            "w1": np.asarray(W1, np.float32),
            "b1": np.asarray(b1, np.float32).reshape(1, 1),
        })
    return nc, in_maps, npc, n_real


def kernel(**inputs):
    nc, in_maps, npc, n_real = _setup(**inputs)
    res = run_bass_kernel_spmd(nc, in_maps, list(range(P)))
    out = np.concatenate([res.results[c]["y"][0, :npc] for c in range(P)])
    return out.reshape(n_real, 1).astype(np.float32)


def run_traced(inputs):
    nc, in_maps, npc, n_real = _setup(**inputs)
    return run_bass_kernel_spmd(nc, in_maps, list(range(P)), trace=True)


# revision 16
# speedup vs baseline: 1.0333x; 1.0333x over previous
"""TAGConvNet (2x TAGConv K=3 + MLP) on 8 trn2 NeuronCores via Bass/Tile.

v2: flipped-orientation bf16 message passing.
- Node-partition across 8 cores (12544 padded rows each, 98 blocks of 128).
- Table = x_k row-major bf16 [NTOT,128] (replicated via AllGather each hop).
- Per hop: dma_gather rows of the table (bf16), scatter via matmul with
  STATIC one-hot tiles (norm_e folded in, loaded from DRAM):
      accT[C, tgt] += msg[e, C]^T-as-lhsT @ oh[e, tgt]
  giving x_{k+1} directly transposed for the dense W matmuls. Table writes
  go through the DMA XBAR transpose. Buckets are padded to 16-idx
  granularity (cross-core max); scatter matmuls are partition-sliced
  "pieces" so chunks may mix adjacent blocks of a 4-block group.
"""
import sys
from contextlib import ExitStack

import numpy as np

sys.path.insert(0, "/opt/trn_rl_repo")

import concourse.bass as bass  # noqa: E402
import concourse.tile as tile  # noqa: E402
from concourse import bacc, mybir  # noqa: E402
from concourse.bass_utils import run_bass_kernel_spmd  # noqa: E402

P = 8                 # cores
NBLK = 98             # 128-node blocks per core
NB = NBLK * 128       # 12544 padded nodes per core
NTOT = P * NB         # 100352
SEG = 25088           # int16-safe gather segment (NTOT / 4)
NSEGS = NTOT // SEG   # 4
GBLK = 4              # blocks per psum group
MAXL = 2048           # max idxs per dma_gather call
PAD_GRAN = 16         # bucket padding granularity (128 = chunk-aligned)
DT = mybir.dt

_cache = {}


def _host_prep(edge_index, n_real):
    """Bucket edges by (core, target block, source segment); pad buckets to
    PAD_GRAN (cross-core common). Returns per-core idx streams, static
    one-hot tiles (norm folded), and the call/piece plan."""
    npc = n_real // P
    row, col = edge_index[0].astype(np.int64), edge_index[1].astype(np.int64)

    deg = np.bincount(col, minlength=n_real)
    dis = np.where(deg > 0, 1.0 / np.sqrt(np.maximum(deg, 1.0)), 0.0).astype(np.float64)
    norm = (dis[row] * dis[col]).astype(np.float32)  # [E]

    def to_gid(i):
        return (i // npc) * NB + (i % npc)

    rg, cg = to_gid(row), to_gid(col)
    core = cg // NB
    loc = cg - core * NB
    blk = loc >> 7
    slot = loc & 127
    seg = rg // SEG

    cnt = np.zeros((P, NBLK, NSEGS), np.int64)
    np.add.at(cnt, (core, blk, seg), 1)
    gran = PAD_GRAN
    pbs = (gran * np.ceil(cnt.max(axis=0) / gran)).astype(np.int64)  # [NBLK, NSEGS]

    # stream layout: per group of GBLK blocks: per seg: buckets back-to-back.
    # calls: split at MAXL (16-granular) within (group, seg).
    off = np.zeros((NBLK, NSEGS), np.int64)
    pos = 0
    groups = [list(range(g, min(g + GBLK, NBLK))) for g in range(0, NBLK, GBLK)]
    calls = []  # (gi, stream_off, L, seg, [(block, off_in_call, len), ...])
    for gi, blocks in enumerate(groups):
        for s in range(NSEGS):
            cur = None
            for b in blocks:
                n = int(pbs[b, s])
                if n == 0:
                    continue
                off[b, s] = pos
                if cur is not None and cur[2] + n <= MAXL:
                    cur[4].append((b, cur[2], n))
                    cur[2] += n
                else:
                    if cur is not None:
                        calls.append(tuple(cur))
                    cur = [gi, pos, n, s, [(b, 0, n)]]
                pos += n
            if cur is not None:
                calls.append(tuple(cur))
    epad = pos

    # per-call pieces; one-hot tiles are per-PIECE (rows outside the piece
    # zeroed) so every matmul uses the full 128-row chunk as lhsT.
    totch = 0
    call_plan = []  # (gi, c_off, L, s, pid0, pieces)
    for (gi, c_off, L, s, bks) in calls:
        nch = -(-L // 128)
        pieces = []  # (ch, r0, r1, b)
        for (b, o, n) in bks:
            p0 = o
            while p0 < o + n:
                ch = p0 // 128
                p1 = min(o + n, (ch + 1) * 128)
                pieces.append((ch, p0 - ch * 128, p1 - ch * 128, b))
                p0 = p1
        call_plan.append((gi, c_off, L, s, totch, pieces))
        totch += len(pieces)

    # first/last piece flags per block (per hop program order)
    first_piece = {}
    last_piece = {}
    for ci, (gi, c_off, L, s, pid0, pieces) in enumerate(call_plan):
        for pi, (ch, r0, r1, b) in enumerate(pieces):
            if b not in first_piece:
                first_piece[b] = (ci, pi)
            last_piece[b] = (ci, pi)
    assert len(first_piece) == NBLK, "every block needs at least one edge bucket"
    # PSUM bank packing: 4 accs per 2KB bank; start=True only on the
    # program-first matmul touching each bank (zeroes the whole bank).
    bank_first = {}
    bank_last = {}
    for gi, blocks in enumerate(groups):
        for bk in range(0, len(blocks), 4):
            bank_blocks = blocks[bk:bk + 4]
            bank_first[(gi, bk // 4)] = min(first_piece[b] for b in bank_blocks)
            bank_last[(gi, bk // 4)] = max(last_piece[b] for b in bank_blocks)

    # per-core idx streams + one-hot tiles
    key = (core * NBLK + blk) * NSEGS + seg
    order = np.argsort(key, kind="stable")
    key_s = key[order]
    first = np.searchsorted(key_s, key_s)
    rank = np.arange(len(key_s)) - first
    dst = off[blk[order], seg[order]] + rank  # stream position

    # pad the idx stream so every call can gather whole 128-row chunks
    # (rows past a call's L are oh-masked; extra idxs read the next call's
    # stream region, which is always a valid row of the current segment)
    epad_pad = epad + MAXL
    gidx = np.zeros((P, epad_pad), np.int16)
    gidx[core[order], dst] = (rg[order] - seg[order] * SEG).astype(np.int16)
    idx16 = np.tile(gidx.reshape(P, epad_pad // 16, 16).transpose(0, 2, 1),
                    (1, 8, 1)).copy()

    # one-hot tiles [128, totpieces, 128] per core: oh[row, pid, slot] = norm
    # for rows within the piece's [r0, r1) chunk-row range, zero elsewhere.
    pos2pid = np.zeros(epad, np.int64)
    pos2row = np.zeros(epad, np.int64)
    for (gi, c_off, L, s, pid0, pieces) in call_plan:
        for pi, (ch, r0, r1, b) in enumerate(pieces):
            a = c_off + ch * 128 + r0
            n = r1 - r0
            pos2pid[a:a + n] = pid0 + pi
            pos2row[a:a + n] = np.arange(r0, r1)
    oh = np.zeros((P, 128, totch, 128), np.float32)
    oh[core[order], pos2row[dst], pos2pid[dst], slot[order]] = norm[order]

    return dict(epad=epad_pad, call_plan=call_plan, idx16=idx16, oh=oh,
                first_piece=first_piece, last_piece=last_piece,
                bank_first=bank_first, bank_last=bank_last,
                groups=groups, totch=totch, npc=npc)


def _build(prep, n_g, k_hops, n_m):
    epad = prep["epad"]
    call_plan = prep["call_plan"]
    groups = prep["groups"]
    totch = prep["totch"]
    first_piece, last_piece = prep["first_piece"], prep["last_piece"]
    bank_first = prep["bank_first"]
    bank_last = prep["bank_last"]
    max_pieces = max(len(p[5]) for p in call_plan)
    nm1 = k_hops + 1

    nc = bacc.Bacc("TRN2", target_bir_lowering=False, debug=False, num_devices=P)

    xT_d = nc.dram_tensor("xT", [8, NB], DT.float32, kind="ExternalInput")
    idx_d = nc.dram_tensor("idx", [128, epad // 16], DT.int16, kind="ExternalInput")
    oh_d = nc.dram_tensor("oh", [128, totch, 128], DT.bfloat16, kind="ExternalInput")
    w0_d = nc.dram_tensor("w0", [8, 128], DT.float32, kind="ExternalInput")
    b0_d = nc.dram_tensor("b0", [128, 1], DT.float32, kind="ExternalInput")
    wtag_d = nc.dram_tensor("wtag", [n_g * nm1, 128, 128], DT.float32, kind="ExternalInput")
    wtagb_d = nc.dram_tensor("wtagb", [n_g * nm1, 128, 128], DT.bfloat16, kind="ExternalInput")
    btag_d = nc.dram_tensor("btag", [128, n_g], DT.float32, kind="ExternalInput")
    wmlp_d = nc.dram_tensor("wmlp", [n_m, 128, 128], DT.float32, kind="ExternalInput")
    bmlp_d = nc.dram_tensor("bmlp", [128, n_m], DT.float32, kind="ExternalInput")
    w1_d = nc.dram_tensor("w1", [128, 1], DT.float32, kind="ExternalInput")
    b1_d = nc.dram_tensor("b1", [1, 1], DT.float32, kind="ExternalInput")
    y_d = nc.dram_tensor("y", [1, NB], DT.float32, kind="ExternalOutput")

    xin = [nc.dram_tensor(f"xin{i}", [NB, 128], DT.bfloat16) for i in range(2)]
    xtab = [nc.dram_tensor(f"xtab{i}", [NTOT, 128], DT.bfloat16, addr_space="Shared")
            for i in range(2)]
    rg = [list(range(P))]

    with tile.TileContext(nc) as tc:
        with ExitStack() as ctx:
            const = ctx.enter_context(tc.tile_pool(name="const", bufs=1))
            big = ctx.enter_context(tc.tile_pool(name="big", bufs=1))
            mpool = ctx.enter_context(tc.tile_pool(name="msg", bufs=3))
            opool = ctx.enter_context(tc.tile_pool(name="oh", bufs=2))
            wpool = ctx.enter_context(tc.tile_pool(name="work", bufs=2))
            tpool = ctx.enter_context(tc.tile_pool(name="tr", bufs=2))
            pacc = ctx.enter_context(tc.tile_pool(name="pacc", bufs=1, space="PSUM"))
            pden = ctx.enter_context(tc.tile_pool(name="pden", bufs=2, space="PSUM"))

            idx_sb = const.tile([128, epad // 16], DT.int16)
            nc.sync.dma_start(idx_sb[:], idx_d[:])

            w0_sb = const.tile([8, 128], DT.float32)
            nc.sync.dma_start(w0_sb[:], w0_d[:])
            b0_sb = const.tile([128, 1], DT.float32)
            nc.sync.dma_start(b0_sb[:], b0_d[:])
            wtagb_sb = []
            for i in range(n_g * nm1):
                tb = const.tile([128, 128], DT.bfloat16, tag=f"wtagb{i}")
                nc.sync.dma_start(tb[:], wtagb_d[i])
                wtagb_sb.append(tb)
            btag_sb = const.tile([128, n_g], DT.float32)
            nc.sync.dma_start(btag_sb[:], btag_d[:])
            wmlp_sb = []
            for i in range(n_m):
                t = const.tile([128, 128], DT.float32, tag=f"wmlp{i}")
                nc.sync.dma_start(t[:], wmlp_d[i])
                wmlp_sb.append(t)
            bmlp_sb = const.tile([128, n_m], DT.float32)
            nc.sync.dma_start(bmlp_sb[:], bmlp_d[:])
            w1_sb = const.tile([128, 1], DT.float32)
            nc.sync.dma_start(w1_sb[:], w1_d[:])
            b1_sb = const.tile([1, 1], DT.float32)
            nc.sync.dma_start(b1_sb[:], b1_d[:])

            hT = big.tile([128, NB], DT.float32)    # h transposed [C, nodes]
            oT = big.tile([128, NB], DT.float32)    # out accumulator

            cpy = mybir.ActivationFunctionType.Copy
            rel = mybir.ActivationFunctionType.Relu

            def write_table(src_sb, slot, need_convert):
                """src_sb [128, NB] (fp32 if need_convert else bf16) ->
                xin[slot] row-major bf16 -> AllGather -> xtab[slot]."""
                for bb in range(0, NBLK, 4):
                    nb4 = min(4, NBLK - bb)
                    w = nb4 * 128
                    sl = src_sb[:, 128 * bb:128 * bb + w]
                    if need_convert:
                        cb = wpool.tile([128, 512], DT.bfloat16, tag="cb")
                        nc.scalar.activation(cb[:, :w], sl, cpy)
                        sl = cb[:, :w]
                    tr = tpool.tile([128, GBLK, 128], DT.bfloat16, tag="tr")
                    nc.sync.dma_start_transpose(tr[:, :nb4, :], sl)
                    dst = xin[slot][128 * bb:128 * bb + w, :].rearrange(
                        "(c p) f -> p c f", p=128)
                    nc.sync.dma_start(dst, tr[:, :nb4, :])
                nc.gpsimd.collective_compute(
                    "AllGather", mybir.AluOpType.bypass, replica_groups=rg,
                    ins=[xin[slot][:]], outs=[xtab[slot][:]])

            # ---- lin0: hT = relu(W0^T xT + b0) ----
            for bb in range(0, NBLK, 4):
                w = min(4, NBLK - bb) * 128
                xt = wpool.tile([8, 512], DT.float32, tag="xt")
                nc.sync.dma_start(xt[:, :w], xT_d[:, 128 * bb:128 * bb + w])
                ph = pden.tile([128, 512], DT.float32, tag="ph")
                nc.tensor.matmul(ph[:, :w], w0_sb[:], xt[:, :w])
                nc.scalar.activation(hT[:, 128 * bb:128 * bb + w], ph[:, :w],
                                     rel, bias=b0_sb[:])

            par = 0
            write_table(hT, par, need_convert=True)

            for g in range(n_g):
                # out = W[g,0]^T h (bf16 weights; convert h slices)
                for bb in range(0, NBLK, 4):
                    w = min(4, NBLK - bb) * 128
                    hb = wpool.tile([128, 512], DT.bfloat16, tag="cb")
                    nc.scalar.activation(hb[:, :w], hT[:, 128 * bb:128 * bb + w], cpy)
                    po = pden.tile([128, 512], DT.float32, tag="ph")
                    nc.tensor.matmul(po[:, :w], wtagb_sb[g * nm1][:], hb[:, :w])
                    nc.vector.tensor_copy(oT[:, 128 * bb:128 * bb + w], po[:, :w])

                for k in range(1, k_hops + 1):
                    nxt = par ^ 1
                    ci = 0
                    for gi, blocks in enumerate(groups):
                        accs = {b: pacc.tile([128, 128], DT.float32,
                                             name=f"acc_{g}_{k}_{b}",
                                             tag=f"acc{b - blocks[0]}")
                                for b in blocks}

                        def acc_ap(b):
                            return accs[b][:]

                        while ci < len(call_plan) and call_plan[ci][0] == gi:
                            (_, c_off, L, s, pid0, pieces) = call_plan[ci]
                            nch = -(-L // 128)
                            npc_ = len(pieces)
                            msg = mpool.tile([128, MAXL // 128, 128], DT.bfloat16,
                                             tag="msg")
                            lg = nch * 128
                            nc.gpsimd.dma_gather(
                                out_ap=msg[:, :nch, :],
                                in_ap=xtab[par][s * SEG:(s + 1) * SEG, :],
                                idxs_ap=idx_sb[:, c_off // 16:(c_off + lg) // 16],
                                num_idxs=lg, num_idxs_reg=lg, elem_size=128)
                            oht = opool.tile([128, max_pieces, 128],
                                             DT.bfloat16, tag="oht")
                            nc.sync.dma_start(oht[:, :npc_, :],
                                              oh_d[:, pid0:pid0 + npc_, :])
                            for pi, (ch, r0, r1, b) in enumerate(pieces):
                                nc.tensor.matmul(
                                    acc_ap(b),
                                    msg[:, ch, :], oht[:, pi, :],
                                    start=(first_piece[b] == (ci, pi)),
                                    stop=(last_piece[b] == (ci, pi)))
                            ci += 1
                        # group finalize: xcur tile, dense W_k, table write
                        bb = blocks[0]
                        w = len(blocks) * 128
                        xc = wpool.tile([128, GBLK * 128], DT.bfloat16, tag="xc")
                        for j, b in enumerate(blocks):
                            nc.scalar.activation(xc[:, 128 * j:128 * (j + 1)],
                                                 acc_ap(b), cpy)
                        for dd in range(0, w, 512):
                            dw = min(512, w - dd)
                            po = pden.tile([128, 512], DT.float32, tag="ph")
                            nc.tensor.matmul(po[:, :dw], wtagb_sb[g * nm1 + k][:],
                                             xc[:, dd:dd + dw])
                            nc.vector.tensor_add(
                                oT[:, 128 * bb + dd:128 * bb + dd + dw],
                                oT[:, 128 * bb + dd:128 * bb + dd + dw], po[:, :dw])
                        if k < k_hops:
                            tr = tpool.tile([128, GBLK, 128], DT.bfloat16, tag="tr")
                            nc.sync.dma_start_transpose(
                                tr[:, :len(blocks), :], xc[:, :w])
                            dst = xin[nxt][128 * bb:128 * bb + w, :].rearrange(
                                "(c p) f -> p c f", p=128)
                            nc.sync.dma_start(dst, tr[:, :len(blocks), :])
                    if k < k_hops:
                        nc.gpsimd.collective_compute(
                            "AllGather", mybir.AluOpType.bypass, replica_groups=rg,
                            ins=[xin[nxt][:]], outs=[xtab[nxt][:]])
                        par = nxt

                # layer end: h = relu(out + b_tag[g])
                nc.scalar.activation(oT[:], oT[:], rel, bias=btag_sb[:, g:g + 1])
                hT, oT = oT, hT
                if g < n_g - 1:
                    nxt = par ^ 1
                    write_table(hT, nxt, need_convert=True)
                    par = nxt

            # ---- MLP ----
            for m in range(n_m):
                for bb in range(0, NBLK, 4):
                    w = min(4, NBLK - bb) * 128
                    po = pden.tile([128, 512], DT.float32, tag="ph")
                    nc.tensor.matmul(po[:, :w], wmlp_sb[m][:],
                                     hT[:, 128 * bb:128 * bb + w])
                    nc.scalar.activation(oT[:, 128 * bb:128 * bb + w], po[:, :w],
                                         rel, bias=bmlp_sb[:, m:m + 1])
                hT, oT = oT, hT

            # ---- head ----
            ysb = big.tile([1, NB], DT.float32)
            for bb in range(0, NBLK, 4):
                w = min(4, NBLK - bb) * 128
                py = pden.tile([128, 512], DT.float32, tag="ph")
                nc.tensor.matmul(py[:1, :w], w1_sb[:], hT[:, 128 * bb:128 * bb + w])
                nc.scalar.activation(ysb[:, 128 * bb:128 * bb + w], py[:1, :w],
                                     rel, bias=b1_sb[:])
            nc.sync.dma_start(y_d[:], ysb[:])

    nc.compile()
    return nc


def _setup(x, edge_index, W0, b0, W_tag, b_tag, W_mlp, b_mlp, W1, b1):
    import jax.numpy as jnp
    x = np.asarray(x, np.float32)
    edge_index = np.asarray(edge_index)
    n_real = x.shape[0]
    n_g, nm1 = W_tag.shape[0], W_tag.shape[1]
    n_m = W_mlp.shape[0]

    ck = (n_real, edge_index.shape[1], int(edge_index[0, ::997].astype(np.int64).sum()),
          int(edge_index[1, ::997].astype(np.int64).sum()))
    if ck not in _cache:
        prep = _host_prep(edge_index, n_real)
        nc = _build(prep, n_g, nm1 - 1, n_m)
        oh_bf = np.asarray(jnp.asarray(prep["oh"], dtype=jnp.bfloat16))
        _cache[ck] = (prep, nc, oh_bf)
    prep, nc, oh_bf = _cache[ck]

    npc = prep["npc"]
    xT = np.zeros((P, 8, NB), np.float32)
    xs = x.reshape(P, npc, -1)
    for c in range(P):
        xT[c, :xs.shape[2], :npc] = xs[c].T

    wtag = np.ascontiguousarray(W_tag.reshape(n_g * nm1, 128, 128), dtype=np.float32)
    wtagb = np.asarray(jnp.asarray(wtag, dtype=jnp.bfloat16))
    in_maps = []
    for c in range(P):
        in_maps.append({
            "xT": xT[c], "idx": prep["idx16"][c], "oh": oh_bf[c],
            "w0": np.vstack([np.asarray(W0, np.float32),
                             np.zeros((8 - W0.shape[0], 128), np.float32)]),
            "b0": np.asarray(b0, np.float32).reshape(128, 1),
            "wtag": wtag, "wtagb": wtagb,
            "btag": np.ascontiguousarray(np.asarray(b_tag, np.float32).T),
            "wmlp": np.asarray(W_mlp, np.float32),
            "bmlp": np.ascontiguousarray(np.asarray(b_mlp, np.float32).T),
            "w1": np.asarray(W1, np.float32),
            "b1": np.asarray(b1, np.float32).reshape(1, 1),
        })
    return nc, in_maps, npc, n_real


def kernel(**inputs):
    nc, in_maps, npc, n_real = _setup(**inputs)
    res = run_bass_kernel_spmd(nc, in_maps, list(range(P)))
    out = np.concatenate([res.results[c]["y"][0, :npc] for c in range(P)])
    return out.reshape(n_real, 1).astype(np.float32)


def run_traced(inputs):
    nc, in_maps, npc, n_real = _setup(**inputs)
    return run_bass_kernel_spmd(nc, in_maps, list(range(P)), trace=True)


# revision 17
# speedup vs baseline: 1.0366x; 1.0031x over previous
"""TAGConvNet (2x TAGConv K=3 + MLP) on 8 trn2 NeuronCores via Bass/Tile.

v2: flipped-orientation bf16 message passing.
- Node-partition across 8 cores (12544 padded rows each, 98 blocks of 128).
- Table = x_k row-major bf16 [NTOT,128] (replicated via AllGather each hop).
- Per hop: dma_gather rows of the table (bf16), scatter via matmul with
  STATIC one-hot tiles (norm_e folded in, loaded from DRAM):
      accT[C, tgt] += msg[e, C]^T-as-lhsT @ oh[e, tgt]
  giving x_{k+1} directly transposed for the dense W matmuls. Table writes
  go through the DMA XBAR transpose. Buckets are padded to 16-idx
  granularity (cross-core max); scatter matmuls are partition-sliced
  "pieces" so chunks may mix adjacent blocks of a 4-block group.
"""
import sys
from contextlib import ExitStack

import numpy as np

sys.path.insert(0, "/opt/trn_rl_repo")

import concourse.bass as bass  # noqa: E402
import concourse.tile as tile  # noqa: E402
from concourse import bacc, mybir  # noqa: E402
from concourse.bass_utils import run_bass_kernel_spmd  # noqa: E402

P = 8                 # cores
NBLK = 98             # 128-node blocks per core
NB = NBLK * 128       # 12544 padded nodes per core
NTOT = P * NB         # 100352
SEG = 25088           # int16-safe gather segment (NTOT / 4)
NSEGS = NTOT // SEG   # 4
GBLK = 4              # blocks per psum group
MAXL = 2048           # max idxs per dma_gather call
PAD_GRAN = 16         # bucket padding granularity (128 = chunk-aligned)
DT = mybir.dt

_cache = {}


def _host_prep(edge_index, n_real):
    """Bucket edges by (core, target block, source segment); pad buckets to
    PAD_GRAN (cross-core common). Returns per-core idx streams, static
    one-hot tiles (norm folded), and the call/piece plan."""
    npc = n_real // P
    row, col = edge_index[0].astype(np.int64), edge_index[1].astype(np.int64)

    deg = np.bincount(col, minlength=n_real)
    dis = np.where(deg > 0, 1.0 / np.sqrt(np.maximum(deg, 1.0)), 0.0).astype(np.float64)
    norm = (dis[row] * dis[col]).astype(np.float32)  # [E]

    def to_gid(i):
        return (i // npc) * NB + (i % npc)

    rg, cg = to_gid(row), to_gid(col)
    core = cg // NB
    loc = cg - core * NB
    blk = loc >> 7
    slot = loc & 127
    seg = rg // SEG

    cnt = np.zeros((P, NBLK, NSEGS), np.int64)
    np.add.at(cnt, (core, blk, seg), 1)
    gran = PAD_GRAN
    pbs = (gran * np.ceil(cnt.max(axis=0) / gran)).astype(np.int64)  # [NBLK, NSEGS]

    # stream layout: per group of GBLK blocks: per seg: buckets back-to-back.
    # calls: split at MAXL (16-granular) within (group, seg).
    off = np.zeros((NBLK, NSEGS), np.int64)
    pos = 0
    groups = [list(range(g, min(g + GBLK, NBLK))) for g in range(0, NBLK, GBLK)]
    calls = []  # (gi, stream_off, L, seg, [(block, off_in_call, len), ...])
    for gi, blocks in enumerate(groups):
        for s in range(NSEGS):
            cur = None
            for b in blocks:
                n = int(pbs[b, s])
                if n == 0:
                    continue
                off[b, s] = pos
                if cur is not None and cur[2] + n <= MAXL:
                    cur[4].append((b, cur[2], n))
                    cur[2] += n
                else:
                    if cur is not None:
                        calls.append(tuple(cur))
                    cur = [gi, pos, n, s, [(b, 0, n)]]
                pos += n
            if cur is not None:
                calls.append(tuple(cur))
    epad = pos

    # per-call pieces; one-hot tiles are per-PIECE (rows outside the piece
    # zeroed) so every matmul uses the full 128-row chunk as lhsT.
    totch = 0
    call_plan = []  # (gi, c_off, L, s, pid0, pieces)
    for (gi, c_off, L, s, bks) in calls:
        nch = -(-L // 128)
        pieces = []  # (ch, r0, r1, b)
        for (b, o, n) in bks:
            p0 = o
            while p0 < o + n:
                ch = p0 // 128
                p1 = min(o + n, (ch + 1) * 128)
                pieces.append((ch, p0 - ch * 128, p1 - ch * 128, b))
                p0 = p1
        call_plan.append((gi, c_off, L, s, totch, pieces))
        totch += len(pieces)

    # first/last piece flags per block (per hop program order)
    first_piece = {}
    last_piece = {}
    for ci, (gi, c_off, L, s, pid0, pieces) in enumerate(call_plan):
        for pi, (ch, r0, r1, b) in enumerate(pieces):
            if b not in first_piece:
                first_piece[b] = (ci, pi)
            last_piece[b] = (ci, pi)
    assert len(first_piece) == NBLK, "every block needs at least one edge bucket"
    # PSUM bank packing: 4 accs per 2KB bank; start=True only on the
    # program-first matmul touching each bank (zeroes the whole bank).
    bank_first = {}
    bank_last = {}
    for gi, blocks in enumerate(groups):
        for bk in range(0, len(blocks), 4):
            bank_blocks = blocks[bk:bk + 4]
            bank_first[(gi, bk // 4)] = min(first_piece[b] for b in bank_blocks)
            bank_last[(gi, bk // 4)] = max(last_piece[b] for b in bank_blocks)

    # per-core idx streams + one-hot tiles
    key = (core * NBLK + blk) * NSEGS + seg
    order = np.argsort(key, kind="stable")
    key_s = key[order]
    first = np.searchsorted(key_s, key_s)
    rank = np.arange(len(key_s)) - first
    dst = off[blk[order], seg[order]] + rank  # stream position

    # pad the idx stream so every call can gather whole 128-row chunks
    # (rows past a call's L are oh-masked; extra idxs read the next call's
    # stream region, which is always a valid row of the current segment)
    epad_pad = epad + MAXL
    gidx = np.zeros((P, epad_pad), np.int16)
    gidx[core[order], dst] = (rg[order] - seg[order] * SEG).astype(np.int16)
    idx16 = np.tile(gidx.reshape(P, epad_pad // 16, 16).transpose(0, 2, 1),
                    (1, 8, 1)).copy()

    # one-hot tiles [128, totpieces, 128] per core: oh[row, pid, slot] = norm
    # for rows within the piece's [r0, r1) chunk-row range, zero elsewhere.
    pos2pid = np.zeros(epad, np.int64)
    pos2row = np.zeros(epad, np.int64)
    for (gi, c_off, L, s, pid0, pieces) in call_plan:
        for pi, (ch, r0, r1, b) in enumerate(pieces):
            a = c_off + ch * 128 + r0
            n = r1 - r0
            pos2pid[a:a + n] = pid0 + pi
            pos2row[a:a + n] = np.arange(r0, r1)
    oh = np.zeros((P, 128, totch, 128), np.float32)
    oh[core[order], pos2row[dst], pos2pid[dst], slot[order]] = norm[order]

    return dict(epad=epad_pad, call_plan=call_plan, idx16=idx16, oh=oh,
                first_piece=first_piece, last_piece=last_piece,
                bank_first=bank_first, bank_last=bank_last,
                groups=groups, totch=totch, npc=npc)


def _build(prep, n_g, k_hops, n_m):
    epad = prep["epad"]
    call_plan = prep["call_plan"]
    groups = prep["groups"]
    totch = prep["totch"]
    first_piece, last_piece = prep["first_piece"], prep["last_piece"]
    bank_first = prep["bank_first"]
    bank_last = prep["bank_last"]
    max_pieces = max(len(p[5]) for p in call_plan)
    nm1 = k_hops + 1

    nc = bacc.Bacc("TRN2", target_bir_lowering=False, debug=False, num_devices=P)

    xT_d = nc.dram_tensor("xT", [8, NB], DT.float32, kind="ExternalInput")
    idx_d = nc.dram_tensor("idx", [128, epad // 16], DT.int16, kind="ExternalInput")
    oh_d = nc.dram_tensor("oh", [128, totch, 128], DT.bfloat16, kind="ExternalInput")
    w0_d = nc.dram_tensor("w0", [8, 128], DT.float32, kind="ExternalInput")
    b0_d = nc.dram_tensor("b0", [128, 1], DT.float32, kind="ExternalInput")
    wtag_d = nc.dram_tensor("wtag", [n_g * nm1, 128, 128], DT.float32, kind="ExternalInput")
    wtagb_d = nc.dram_tensor("wtagb", [n_g * nm1, 128, 128], DT.bfloat16, kind="ExternalInput")
    btag_d = nc.dram_tensor("btag", [128, n_g], DT.float32, kind="ExternalInput")
    wmlp_d = nc.dram_tensor("wmlp", [n_m, 128, 128], DT.float32, kind="ExternalInput")
    bmlp_d = nc.dram_tensor("bmlp", [128, n_m], DT.float32, kind="ExternalInput")
    w1_d = nc.dram_tensor("w1", [128, 1], DT.float32, kind="ExternalInput")
    b1_d = nc.dram_tensor("b1", [1, 1], DT.float32, kind="ExternalInput")
    y_d = nc.dram_tensor("y", [1, NB], DT.float32, kind="ExternalOutput")

    xin = [nc.dram_tensor(f"xin{i}", [NB, 128], DT.bfloat16) for i in range(2)]
    xtab = [nc.dram_tensor(f"xtab{i}", [NTOT, 128], DT.bfloat16, addr_space="Shared")
            for i in range(2)]
    rg = [list(range(P))]

    with tile.TileContext(nc) as tc:
        with ExitStack() as ctx:
            const = ctx.enter_context(tc.tile_pool(name="const", bufs=1))
            big = ctx.enter_context(tc.tile_pool(name="big", bufs=1))
            mpool = ctx.enter_context(tc.tile_pool(name="msg", bufs=3))
            opool = ctx.enter_context(tc.tile_pool(name="oh", bufs=2))
            wpool = ctx.enter_context(tc.tile_pool(name="work", bufs=2))
            tpool = ctx.enter_context(tc.tile_pool(name="tr", bufs=2))
            pacc = ctx.enter_context(tc.tile_pool(name="pacc", bufs=1, space="PSUM"))
            pden = ctx.enter_context(tc.tile_pool(name="pden", bufs=2, space="PSUM"))

            idx_sb = const.tile([128, epad // 16], DT.int16)
            nc.sync.dma_start(idx_sb[:], idx_d[:])

            w0_sb = const.tile([8, 128], DT.float32)
            nc.sync.dma_start(w0_sb[:], w0_d[:])
            b0_sb = const.tile([128, 1], DT.float32)
            nc.sync.dma_start(b0_sb[:], b0_d[:])
            wtagb_sb = []
            for i in range(n_g * nm1):
                tb = const.tile([128, 128], DT.bfloat16, tag=f"wtagb{i}")
                nc.sync.dma_start(tb[:], wtagb_d[i])
                wtagb_sb.append(tb)
            btag_sb = const.tile([128, n_g], DT.float32)
            nc.sync.dma_start(btag_sb[:], btag_d[:])
            wmlp_sb = []
            for i in range(n_m):
                t = const.tile([128, 128], DT.float32, tag=f"wmlp{i}")
                nc.sync.dma_start(t[:], wmlp_d[i])
                wmlp_sb.append(t)
            bmlp_sb = const.tile([128, n_m], DT.float32)
            nc.sync.dma_start(bmlp_sb[:], bmlp_d[:])
            w1_sb = const.tile([128, 1], DT.float32)
            nc.sync.dma_start(w1_sb[:], w1_d[:])
            b1_sb = const.tile([1, 1], DT.float32)
            nc.sync.dma_start(b1_sb[:], b1_d[:])

            hT = big.tile([128, NB], DT.float32)    # h transposed [C, nodes]
            oT = big.tile([128, NB], DT.float32)    # out accumulator
            for _ in range(3):
                mz = mpool.tile([128, MAXL // 128, 128], DT.bfloat16, tag="msg")
                nc.vector.memset(mz[:], 0.0)

            cpy = mybir.ActivationFunctionType.Copy
            rel = mybir.ActivationFunctionType.Relu

            def write_table(src_sb, slot, need_convert):
                """src_sb [128, NB] (fp32 if need_convert else bf16) ->
                xin[slot] row-major bf16 -> AllGather -> xtab[slot]."""
                for bb in range(0, NBLK, 4):
                    nb4 = min(4, NBLK - bb)
                    w = nb4 * 128
                    sl = src_sb[:, 128 * bb:128 * bb + w]
                    if need_convert:
                        cb = wpool.tile([128, 512], DT.bfloat16, tag="cb")
                        nc.scalar.activation(cb[:, :w], sl, cpy)
                        sl = cb[:, :w]
                    tr = tpool.tile([128, GBLK, 128], DT.bfloat16, tag="tr")
                    nc.sync.dma_start_transpose(tr[:, :nb4, :], sl)
                    dst = xin[slot][128 * bb:128 * bb + w, :].rearrange(
                        "(c p) f -> p c f", p=128)
                    nc.sync.dma_start(dst, tr[:, :nb4, :])
                nc.gpsimd.collective_compute(
                    "AllGather", mybir.AluOpType.bypass, replica_groups=rg,
                    ins=[xin[slot][:]], outs=[xtab[slot][:]])

            # ---- lin0: hT = relu(W0^T xT + b0) ----
            for bb in range(0, NBLK, 4):
                w = min(4, NBLK - bb) * 128
                xt = wpool.tile([8, 512], DT.float32, tag="xt")
                nc.sync.dma_start(xt[:, :w], xT_d[:, 128 * bb:128 * bb + w])
                ph = pden.tile([128, 512], DT.float32, tag="ph")
                nc.tensor.matmul(ph[:, :w], w0_sb[:], xt[:, :w])
                nc.scalar.activation(hT[:, 128 * bb:128 * bb + w], ph[:, :w],
                                     rel, bias=b0_sb[:])

            par = 0
            write_table(hT, par, need_convert=True)

            for g in range(n_g):
                # out = W[g,0]^T h (bf16 weights; convert h slices)
                for bb in range(0, NBLK, 4):
                    w = min(4, NBLK - bb) * 128
                    hb = wpool.tile([128, 512], DT.bfloat16, tag="cb")
                    nc.scalar.activation(hb[:, :w], hT[:, 128 * bb:128 * bb + w], cpy)
                    po = pden.tile([128, 512], DT.float32, tag="ph")
                    nc.tensor.matmul(po[:, :w], wtagb_sb[g * nm1][:], hb[:, :w])
                    nc.vector.tensor_copy(oT[:, 128 * bb:128 * bb + w], po[:, :w])

                for k in range(1, k_hops + 1):
                    nxt = par ^ 1
                    ci = 0
                    for gi, blocks in enumerate(groups):
                        accs = {b: pacc.tile([128, 128], DT.float32,
                                             name=f"acc_{g}_{k}_{b}",
                                             tag=f"acc{b - blocks[0]}")
                                for b in blocks}

                        def acc_ap(b):
                            return accs[b][:]

                        while ci < len(call_plan) and call_plan[ci][0] == gi:
                            (_, c_off, L, s, pid0, pieces) = call_plan[ci]
                            nch = -(-L // 128)
                            npc_ = len(pieces)
                            msg = mpool.tile([128, MAXL // 128, 128], DT.bfloat16,
                                             tag="msg")
                            nc.gpsimd.dma_gather(
                                out_ap=msg[:, :nch, :],
                                in_ap=xtab[par][s * SEG:(s + 1) * SEG, :],
                                idxs_ap=idx_sb[:, c_off // 16:(c_off + L) // 16],
                                num_idxs=L, num_idxs_reg=L, elem_size=128)
                            oht = opool.tile([128, max_pieces, 128],
                                             DT.bfloat16, tag="oht")
                            nc.sync.dma_start(oht[:, :npc_, :],
                                              oh_d[:, pid0:pid0 + npc_, :])
                            for pi, (ch, r0, r1, b) in enumerate(pieces):
                                nc.tensor.matmul(
                                    acc_ap(b),
                                    msg[:, ch, :], oht[:, pi, :],
                                    start=(first_piece[b] == (ci, pi)),
                                    stop=(last_piece[b] == (ci, pi)))
                            ci += 1
                        # group finalize: xcur tile, dense W_k, table write
                        bb = blocks[0]
                        w = len(blocks) * 128
                        xc = wpool.tile([128, GBLK * 128], DT.bfloat16, tag="xc")
                        for j, b in enumerate(blocks):
                            nc.scalar.activation(xc[:, 128 * j:128 * (j + 1)],
                                                 acc_ap(b), cpy)
                        for dd in range(0, w, 512):
                            dw = min(512, w - dd)
                            po = pden.tile([128, 512], DT.float32, tag="ph")
                            nc.tensor.matmul(po[:, :dw], wtagb_sb[g * nm1 + k][:],
                                             xc[:, dd:dd + dw])
                            nc.vector.tensor_add(
                                oT[:, 128 * bb + dd:128 * bb + dd + dw],
                                oT[:, 128 * bb + dd:128 * bb + dd + dw], po[:, :dw])
                        if k < k_hops:
                            tr = tpool.tile([128, GBLK, 128], DT.bfloat16, tag="tr")
                            nc.sync.dma_start_transpose(
                                tr[:, :len(blocks), :], xc[:, :w])
                            dst = xin[nxt][128 * bb:128 * bb + w, :].rearrange(
                                "(c p) f -> p c f", p=128)
                            nc.sync.dma_start(dst, tr[:, :len(blocks), :])
                    if k < k_hops:
                        nc.gpsimd.collective_compute(
                            "AllGather", mybir.AluOpType.bypass, replica_groups=rg,
                            ins=[xin[nxt][:]], outs=[xtab[nxt][:]])
                        par = nxt

                # layer end: h = relu(out + b_tag[g])
                nc.scalar.activation(oT[:], oT[:], rel, bias=btag_sb[:, g:g + 1])
                hT, oT = oT, hT
                if g < n_g - 1:
                    nxt = par ^ 1
                    write_table(hT, nxt, need_convert=True)
                    par = nxt

            # ---- MLP ----
            for m in range(n_m):
                for bb in range(0, NBLK, 4):
                    w = min(4, NBLK - bb) * 128
                    po = pden.tile([128, 512], DT.float32, tag="ph")
                    nc.tensor.matmul(po[:, :w], wmlp_sb[m][:],
                                     hT[:, 128 * bb:128 * bb + w])
                    nc.scalar.activation(oT[:, 128 * bb:128 * bb + w], po[:, :w],
                                         rel, bias=bmlp_sb[:, m:m + 1])
                hT, oT = oT, hT

            # ---- head ----
            ysb = big.tile([1, NB], DT.float32)
            for bb in range(0, NBLK, 4):
                w = min(4, NBLK - bb) * 128
                py = pden.tile([128, 512], DT.float32, tag="ph")
                nc.tensor.matmul(py[:1, :w], w1_sb[:], hT[:, 128 * bb:128 * bb + w])
                nc.scalar.activation(ysb[:, 128 * bb:128 * bb + w], py[:1, :w],
                                     rel, bias=b1_sb[:])
            nc.sync.dma_start(y_d[:], ysb[:])

    nc.compile()
    return nc


def _setup(x, edge_index, W0, b0, W_tag, b_tag, W_mlp, b_mlp, W1, b1):
    import jax.numpy as jnp
    x = np.asarray(x, np.float32)
    edge_index = np.asarray(edge_index)
    n_real = x.shape[0]
    n_g, nm1 = W_tag.shape[0], W_tag.shape[1]
    n_m = W_mlp.shape[0]

    ck = (n_real, edge_index.shape[1], int(edge_index[0, ::997].astype(np.int64).sum()),
          int(edge_index[1, ::997].astype(np.int64).sum()))
    if ck not in _cache:
        prep = _host_prep(edge_index, n_real)
        nc = _build(prep, n_g, nm1 - 1, n_m)
        oh_bf = np.asarray(jnp.asarray(prep["oh"], dtype=jnp.bfloat16))
        _cache[ck] = (prep, nc, oh_bf)
    prep, nc, oh_bf = _cache[ck]

    npc = prep["npc"]
    xT = np.zeros((P, 8, NB), np.float32)
    xs = x.reshape(P, npc, -1)
    for c in range(P):
        xT[c, :xs.shape[2], :npc] = xs[c].T

    wtag = np.ascontiguousarray(W_tag.reshape(n_g * nm1, 128, 128), dtype=np.float32)
    wtagb = np.asarray(jnp.asarray(wtag, dtype=jnp.bfloat16))
    in_maps = []
    for c in range(P):
        in_maps.append({
            "xT": xT[c], "idx": prep["idx16"][c], "oh": oh_bf[c],
            "w0": np.vstack([np.asarray(W0, np.float32),
                             np.zeros((8 - W0.shape[0], 128), np.float32)]),
            "b0": np.asarray(b0, np.float32).reshape(128, 1),
            "wtag": wtag, "wtagb": wtagb,
            "btag": np.ascontiguousarray(np.asarray(b_tag, np.float32).T),
            "wmlp": np.asarray(W_mlp, np.float32),
            "bmlp": np.ascontiguousarray(np.asarray(b_mlp, np.float32).T),
            "w1": np.asarray(W1, np.float32),
            "b1": np.asarray(b1, np.float32).reshape(1, 1),
        })
    return nc, in_maps, npc, n_real


def kernel(**inputs):
    nc, in_maps, npc, n_real = _setup(**inputs)
    res = run_bass_kernel_spmd(nc, in_maps, list(range(P)))
    out = np.concatenate([res.results[c]["y"][0, :npc] for c in range(P)])
    return out.reshape(n_real, 1).astype(np.float32)


def run_traced(inputs):
    nc, in_maps, npc, n_real = _setup(**inputs)
    return run_bass_kernel_spmd(nc, in_maps, list(range(P)), trace=True)
